# revision 31
# baseline (speedup 1.0000x reference)
"""Bahdanau-style attention kernel for Trainium2, data-parallel over batch
across 8 NeuronCores.

Reference computation (per batch b):
    e_proj = enc[b] @ We.T            # [S, D]   (We = W[:, 512:], [D, E])
    energy = tanh(e_proj + hidden[b] @ Wh.T + bias)
    scores = energy @ v               # [S]
    attn   = softmax(scores)          # [1, S]

Shapes: B=32, S=2048, E=1024, D=512.  Each core handles 4 batches.

Design notes (hard-won from profiling):
  - enc f32 is DMA-loaded, cast f32->fp16 on VectorE, then transposed via
    the DMA xbar so the contraction dim e lands on partitions.  PE-side
    transposes are ruled out: transpose-mode matmuls don't register as
    PE-busy for the HAM clock gate, so any transpose phase throttles the
    PE to 1.2 GHz (measured 78-122us of throttle in every PE-transpose
    variant).
  - The framework serializes DMA-transposes against ALL other in-flight
    DMAs (HW xbar deadlock erratum), so loads and xbar transposes run
    mutually exclusive no matter which ring they're on.  Everything DMA
    therefore rides ONE ring (sync), self-paced: per unit ~5.9us load +
    ~4.9us xbar.  The other queues are kept strictly decoupled:
    ScalarE = tanh/exp/attn-scale only, DVE = casts only, so no engine
    FIFO ever makes a DMA wait on downstream compute (head-of-line
    blocking measured 15-20us/unit in earlier variants).
  - Weights/small inputs arrive pre-transposed + pre-cast from the host
    (layout transforms only; all FLOPs stay on device).
  - main matmul: psum[d128, s512] += WeT[e128, d128].T @ encT[e128, s512]
    (fp16 -> FWL fast weight load, hidden by the PE's weight-load
    pull-ahead).
  - tanh fused with the (h_proj + b) bias via ScalarE activation
    (per-partition bias, d is the partition dim).
  - scores via TensorE matvec with v, software-pipelined one unit behind
    the energy matmuls so the PE never waits on ScalarE's tanh.
  - softmax without a max pass: softmax is shift-invariant and |scores|
    stays well inside f32 exp range; exp's accum_out gives block sums for
    free, so only a tiny normalize tail remains per batch.
"""

import numpy as np

B, S, E, D = 32, 2048, 1024, 512
N_CORES = 8
BP = B // N_CORES  # batches per core = 4
SBLK = 512  # s-block (psum free dim)
N_SBLK = S // SBLK  # 4
N_ST = SBLK // 128  # 4 s-subtiles per block
N_EC = E // 128  # 8 e-chunks
N_DP = D // 128  # 4 d-chunks
N_KC = D // 128  # 4 k-chunks (hidden proj contraction)
NWC = (D + E) // 128  # 12 column-chunks of W

_CACHE = {}


def _build(debug_dumps=False):
    from contextlib import ExitStack

    import concourse.tile as tile
    from concourse import bacc, mybir

    F32 = mybir.dt.float32
    F16 = mybir.dt.float16
    AF = mybir.ActivationFunctionType
    AX = mybir.AxisListType

    nc = bacc.Bacc("TRN2", target_bir_lowering=False, debug=False,
                   num_devices=N_CORES)

    # Pre-transposed / pre-cast layouts (host prep):
    #   wt[q, dp, c, j]  = W[dp*128+j, c*128+q]   (c<4 -> Wh, c>=4 -> We)
    #   hidt[p, kc, b]   = hidden[b, kc*128+p]
    #   bt[p, dp]        = b[dp*128+p]
    #   vt[p, dp]        = v[dp*128+p]
    wt_d = nc.dram_tensor("wt", [128, N_DP, NWC, 128], F16,
                          kind="ExternalInput").ap()
    hidt_d = nc.dram_tensor("hidt", [128, N_KC, BP], F16,
                            kind="ExternalInput").ap()
    bt_d = nc.dram_tensor("bt", [128, N_DP], F32, kind="ExternalInput").ap()
    vt_d = nc.dram_tensor("vt", [128, N_DP], F16, kind="ExternalInput").ap()
    enc_d = nc.dram_tensor("enc", [BP, S, E], F32, kind="ExternalInput").ap()
    out_d = nc.dram_tensor("out", [BP, S], F32, kind="ExternalOutput").ap()
    if debug_dumps:
        dbg_scores = nc.dram_tensor(
            "dbg_scores", [BP, S], F32, kind="ExternalOutput").ap()

    with tile.TileContext(nc) as tc, ExitStack() as ctx:
        consts = ctx.enter_context(tc.tile_pool(name="consts", bufs=1))
        enc_pool = ctx.enter_context(tc.tile_pool(name="enc", bufs=4))
        enc16_pool = ctx.enter_context(tc.tile_pool(name="enc16", bufs=2))
        enct_pool = ctx.enter_context(tc.tile_pool(name="enct", bufs=2))
        en_pool = ctx.enter_context(tc.tile_pool(name="energy", bufs=2))
        small = ctx.enter_context(tc.tile_pool(name="small", bufs=2))
        ps = ctx.enter_context(tc.tile_pool(name="ps", bufs=6, space="PSUM"))
        ps2 = ctx.enter_context(tc.tile_pool(name="ps2", bufs=2, space="PSUM"))

        def emit_load(bi, sblk):
            enc32 = enc_pool.tile([128, N_ST, E], F32, tag="enc32",
                                  name=f"enc32_{bi}_{sblk}")
            nc.sync.dma_start(
                out=enc32,
                in_=enc_d[bi, sblk * SBLK:(sblk + 1) * SBLK, :].rearrange(
                    "(st p) e -> p st e", p=128
                ),
            )
            return enc32

        def new_pair16(p):
            # one fp16 staging tile per PAIR of units (the xbar transposes
            # two units per call to halve per-call init + mutex handoffs)
            return enc16_pool.tile([128, 2, N_ST, E], F16, tag="enc16",
                                   name=f"enc16_p{p}")

        def emit_cast(enc32, pair16, uu):
            nc.vector.tensor_copy(pair16[:, uu, 0:2, :], enc32[:, 0:2, :])
            nc.vector.tensor_copy(pair16[:, uu, 2:4, :], enc32[:, 2:4, :])

        def emit_xbar_single(pair16, uu, u):
            # single-unit xbar for the prologue: lets unit 0's matmuls
            # start ~14us earlier than waiting for a full 2MB pair call
            enct1 = enct_pool.tile([128, N_ST, N_EC, 128], F16, tag="enct1",
                                   name=f"enct1_{u}")
            nc.sync.dma_start(
                out=enct1.rearrange("p a b j -> p (a b) j"),
                in_=pair16[:, uu].rearrange("p a e -> p (a e)"),
                transpose=True,
            )
            return enct1

        def emit_xbar_pair(pair16, p):
            # fp16 transpose on the DMA xbar, 2 units per call:
            # out[q, (uu, st, ec), j] = pair16[j, uu*4096 + st*1024 + ec*128 + q]
            #   -> enct[q, uu, st, ec, j] = encT[e=ec*128+q,
            #                                    s=(2p+uu)*512 + st*128 + j]
            enct = enct_pool.tile([128, 2, N_ST, N_EC, 128], F16, tag="enct",
                                  name=f"enct_p{p}")
            nc.sync.dma_start(
                out=enct.rearrange("p a b c j -> p (a b c) j"),
                in_=pair16.rearrange("p a b e -> p (a b e)"),
                transpose=True,
            )
            return enct

        # ---- pre-transposed weights & small inputs (contiguous loads) ----
        wt_sb = consts.tile([128, N_DP, NWC, 128], F16)
        nc.sync.dma_start(out=wt_sb, in_=wt_d)
        hidt_sb = consts.tile([128, N_KC, BP], F16)
        nc.sync.dma_start(out=hidt_sb, in_=hidt_d)
        bt_sb = consts.tile([128, N_DP], F32)
        nc.sync.dma_start(out=bt_sb, in_=bt_d)
        vt_sb = consts.tile([128, N_DP], F16)
        nc.sync.dma_start(out=vt_sb, in_=vt_d)

        # prologue: only L0/L1 ahead of the first xbar (the DMA mutex
        # makes every prefetched load's TRANSFER precede the xbar), then
        # L2/L3 behind it.
        LOOKAHEAD = 4  # units of load-emission skew (see main loop)
        pending_loads = {u: emit_load(u // N_SBLK, u % N_SBLK)
                         for u in range(2)}
        pair16s = {0: new_pair16(0)}
        emit_cast(pending_loads[0], pair16s[0], 0)
        enct_u0 = emit_xbar_single(pair16s[0], 0, 0)
        emit_cast(pending_loads[1], pair16s[0], 1)
        enct_u1 = emit_xbar_single(pair16s[0], 1, 1)
        encts = {}
        for u in (2, 3):
            pending_loads[u] = emit_load(u // N_SBLK, u % N_SBLK)
        pair16s[1] = new_pair16(1)
        emit_cast(pending_loads[2], pair16s[1], 0)
        emit_cast(pending_loads[3], pair16s[1], 1)

        # preload the exp/tanh activation table early (overlaps with DMAs)
        warm = consts.tile([1, 1], F32)
        nc.vector.memset(warm, 0.0)
        nc.scalar.activation(warm, warm, AF.Tanh)

        # ---- h_projT + bias -> hbT [128, dp, b] ----
        hbt_sb = consts.tile([128, N_DP, BP], F32)
        for dp in range(N_DP):
            ph = ps2.tile([128, 16], F32, tag="sc")
            for kc in range(N_KC):
                nc.tensor.matmul(
                    ph[:, 0:BP],
                    wt_sb[:, dp, kc, :],
                    hidt_sb[:, kc, :],
                    start=(kc == 0), stop=(kc == N_KC - 1),
                )
            nc.vector.tensor_scalar_add(
                hbt_sb[:, dp, :], ph[:, 0:BP], bt_sb[:, dp:dp + 1]
            )

        # ---- main pipeline over 16 (batch, sblk) units ----
        batch_state = {}

        def emit_scores(bi, sblk, energy):
            """Scores matvec + streamed exp with fused partial sums."""
            psc = ps2.tile([1, SBLK], F32, tag="sc")
            for dp in range(N_DP):
                nc.tensor.matmul(
                    psc, vt_sb[:, dp:dp + 1], energy[:, dp, :],
                    start=(dp == 0), stop=(dp == N_DP - 1),
                )
            if sblk == 0:
                prob = small.tile([1, S], F32, tag="prob")
                partials = small.tile([1, N_SBLK], F32, tag="part")
                batch_state[bi] = (prob, partials)
            prob, partials = batch_state[bi]
            if debug_dumps:
                nc.scalar.copy(
                    dbg_scores_sb[:, sblk * SBLK:(sblk + 1) * SBLK], psc)
            nc.scalar.activation(
                prob[:, sblk * SBLK:(sblk + 1) * SBLK], psc, AF.Exp,
                bias=0.0, scale=1.0,
                accum_out=partials[:, sblk:sblk + 1],
            )
            if sblk == N_SBLK - 1:
                ssum = small.tile([1, 1], F32, tag="ssum")
                nc.vector.reduce_sum(ssum, partials, axis=AX.X)
                rtot = small.tile([1, 1], F32, tag="rtot")
                nc.vector.reciprocal(rtot, ssum)
                attn = small.tile([1, S], F32, tag="attn")
                nc.scalar.activation(attn, prob, AF.Copy, scale=rtot)
                nc.sync.dma_start(out=out_d[bi], in_=attn)
                if debug_dumps:
                    nc.sync.dma_start(
                        out=dbg_scores[bi:bi + 1, :], in_=dbg_scores_sb)
                del batch_state[bi]

        if debug_dumps:
            dbg_scores_sb = small.tile([1, S], F32, tag="dbgsc")

        N_UNITS = BP * N_SBLK
        prev = None  # pending (bi, sblk, energy) for the scores pipeline
        for bi in range(BP):
            for sblk in range(N_SBLK):
                u = bi * N_SBLK + sblk
                un = u + LOOKAHEAD
                if un < N_UNITS:
                    pending_loads[un] = emit_load(un // N_SBLK, un % N_SBLK)
                # casts lead by 4 units so each quad's two xbars can be
                # emitted back-to-back ([4 loads][2 xbars] ring cadence =
                # half the transpose<->DMA mutex transitions)
                c = u + 4
                if c < N_UNITS:
                    p = c // 2
                    if c % 2 == 0:
                        pair16s[p] = new_pair16(p)
                    emit_cast(pending_loads[c], pair16s[p], c % 2)
                if u % 4 == 1:
                    for p in ((u + 1) // 2, (u + 3) // 2):
                        if 1 <= p and 2 * p < N_UNITS and p not in encts:
                            encts[p] = emit_xbar_pair(pair16s[p], p)

                energy = en_pool.tile([128, N_DP, SBLK], F16, tag="energy")
                for dp in range(N_DP):
                    pe = ps.tile([128, SBLK], F32, tag="pe")
                    for ec in range(N_EC):
                        nc.tensor.matmul(
                            pe,
                            wt_sb[:, dp, N_KC + ec, :],
                            (enct_u0 if u == 0 else enct_u1)[:, :, ec, :]
                            if u < 2 else
                            encts[u // 2][:, u % 2, :, ec, :],
                            start=(ec == 0), stop=(ec == N_EC - 1),
                        )
                    nc.scalar.activation(
                        energy[:, dp, :], pe, AF.Tanh,
                        bias=hbt_sb[:, dp, bi:bi + 1], scale=1.0,
                    )
                    # scores for the previous unit, mid-stream so the PE
                    # never waits on ScalarE's tanh of THIS unit
                    if dp == 1 and prev is not None:
                        emit_scores(*prev)
                        prev = None
                prev = (bi, sblk, energy)

        emit_scores(*prev)

    nc.compile()
    return nc


def _get_nc():
    if "nc" not in _CACHE:
        _CACHE["nc"] = _build()
    return _CACHE["nc"]


def make_in_maps(hidden, encoder_outputs, W, b, v):
    """Host-side sharding + weight layout prep (transpose/cast only)."""
    hidden = np.ascontiguousarray(hidden, dtype=np.float32)
    encoder_outputs = np.ascontiguousarray(encoder_outputs, dtype=np.float32)
    W = np.ascontiguousarray(W, dtype=np.float32)
    b = np.ascontiguousarray(b, dtype=np.float32)
    v = np.ascontiguousarray(v, dtype=np.float32)

    # wt[q, dp, c, j] = W[dp*128+j, c*128+q]
    wt = np.ascontiguousarray(
        W.reshape(N_DP, 128, NWC, 128).transpose(3, 0, 2, 1)
    ).astype(np.float16)
    bt = np.ascontiguousarray(b.reshape(N_DP, 128).T)
    vt = np.ascontiguousarray(v.reshape(N_DP, 128).T).astype(np.float16)

    in_maps = []
    for c in range(N_CORES):
        hid_c = hidden[c * BP:(c + 1) * BP]
        # hidt[p, kc, b] = hidden[b, kc*128+p]
        hidt = np.ascontiguousarray(
            hid_c.reshape(BP, N_KC, 128).transpose(2, 1, 0)
        ).astype(np.float16)
        in_maps.append({
            "hidt": hidt,
            "enc": np.ascontiguousarray(encoder_outputs[c * BP:(c + 1) * BP]),
            "wt": wt,
            "bt": bt,
            "vt": vt,
        })
    return in_maps


def kernel(hidden, encoder_outputs, W, b, v):
    from concourse.bass_utils import run_bass_kernel_spmd

    nc = _get_nc()
    in_maps = make_in_maps(hidden, encoder_outputs, W, b, v)
    r = run_bass_kernel_spmd(nc, in_maps, list(range(N_CORES)))
    out = np.concatenate([r.results[c]["out"] for c in range(N_CORES)], axis=0)
    return out[:, None, :].astype(np.float32)
